# revision 17
# baseline (speedup 1.0000x reference)
"""Causal self-attention kernel for Trainium2, sharded over 8 NeuronCores.

Problem: B=4, T=2048, DIM=1024, H=16 heads, head_dim=64, fp32 I/O.

Sharding: (batch, head-group) pairs -> 8 shards. Core c handles batch
b = c//2 and head group g = c%2 (8 heads each). Each core computes its
q/k/v projections for its head slice, causal flash-style attention, and
a partial o_proj against its head-slice of wo. The host sums the two
partial o_proj outputs per batch (the "all-reduce") while gathering.

Layout strategy (per core):
  - Host pre-transposes x and the weight slices so the contraction dim
    (model dim) lands on SBUF partitions, and casts them to bf16.
  - Scores are computed TRANSPOSED: sT[tk, tq] = k @ q^T, so softmax'd
    probabilities come out with tk on partitions -- exactly the layout
    the attn@v matmul needs as its moving operand (lhsT = v).
  - Softmax skips max-subtraction (scores are O(1) by construction:
    q,k ~ N(0,1), dot/8), exp runs on the scalar engine straight out of
    PSUM, and the denominator is obtained for free by augmenting v with
    a ones column.
  - Causal masking inside diagonal 128-tiles is applied by one extra
    accumulating matmul (identity x (-1e9 strictly-lower-tri mask)).
"""

import numpy as np
import ml_dtypes

import concourse.bass as bass
import concourse.bacc as bacc
import concourse.mybir as mybir
import concourse.tile as tile
from concourse.bass import ds, ts
from concourse.bass_utils import run_bass_kernel_spmd
from concourse.masks import make_identity

BF16 = mybir.dt.bfloat16
F32 = mybir.dt.float32

T = 2048
D = 1024
DG = 512          # head-group width (8 heads x 64)
NH = 8            # heads per core
DH = 64
P = 128
NT = T // P       # 16 t-tiles
NKO = D // P      # 8 contraction tiles for projections
NC_CHUNK = 1024   # tq chunk width for attention
NCH = T // NC_CHUNK  # 2 chunks

_CACHED = None  # (nc, input names) -- build/trace once per process

MM_N = 512  # max moving free-dim per matmul instruction


def _mm(nc, out, lhsT, rhs, start, stop, out_off=0):
    """matmul out = lhsT.T @ rhs, sliced so no piece crosses a PSUM bank
    boundary. out_off is the column offset of `out` within its psum tile."""
    n = rhs.shape[-1]
    o = 0
    while o < n:
        w = min(n - o, MM_N - ((out_off + o) % MM_N))
        nc.tensor.matmul(
            out[:, ds(o, w)], lhsT=lhsT, rhs=rhs[:, ds(o, w)],
            start=start, stop=stop,
        )
        o += w


def _build_kernel():
    nc = bacc.Bacc("TRN2", target_bir_lowering=False, debug=False)

    xT_d = nc.dram_tensor("xT", [D, T], BF16, kind="ExternalInput").ap()
    wqT_d = nc.dram_tensor("wqT", [D, DG], BF16, kind="ExternalInput").ap()
    wkT_d = nc.dram_tensor("wkT", [D, DG], BF16, kind="ExternalInput").ap()
    wvT_d = nc.dram_tensor("wvT", [D, DG], BF16, kind="ExternalInput").ap()
    woT_d = nc.dram_tensor("woT", [DG, D], BF16, kind="ExternalInput").ap()
    y_d = nc.dram_tensor("y", [T, D], F32, kind="ExternalOutput").ap()

    with tile.TileContext(nc) as tc:
        with (
            tc.tile_pool(name="const", bufs=1) as const,
            tc.tile_pool(name="sb", bufs=1) as sb,
            tc.tile_pool(name="work", bufs=3) as work,
            tc.tile_pool(name="stgp", bufs=2) as stgp,
            tc.tile_pool(name="ps", bufs=2, space="PSUM") as psp,
            tc.tile_pool(name="av", bufs=2, space="PSUM") as avp,
        ):
            # ---- constants ----
            idn = const.tile([P, P], BF16, tag="idn")
            make_identity(nc, idn)
            msk = const.tile([P, P], BF16, tag="msk")
            # msk[tk, tq] = 0 where tq >= tk else -1e9  (strictly-lower = masked)
            nc.gpsimd.memset(msk, 0.0)
            nc.gpsimd.affine_select(
                out=msk, in_=msk,
                compare_op=mybir.AluOpType.is_ge,
                fill=-1e9, base=0,
                pattern=[[1, P]], channel_multiplier=-1,
            )
            ones64 = const.tile([1, DH], BF16, tag="ones64")
            nc.gpsimd.memset(ones64, 1.0)

            # ---- persistent SBUF tensors ----
            XT = sb.tile([P, NKO, T], BF16, tag="XT")
            WQT = sb.tile([P, NKO, DG], BF16, tag="WQT")
            WKT = sb.tile([P, NKO, DG], BF16, tag="WKT")
            WVT = sb.tile([P, NKO, DG], BF16, tag="WVT")
            WOT = sb.tile([P, DG // P, D], BF16, tag="WOT")
            QT = sb.tile([P, DG // P, T], BF16, tag="QT")
            KT = sb.tile([P, DG // P, T], BF16, tag="KT")
            VA = sb.tile([P, NT, NH, DH + 1], BF16, tag="VA")
            OGT = sb.tile([P, DG // P, T], BF16, tag="OGT")

            # ---- input DMAs (chunked across queues) ----
            xr = xT_d.rearrange("(ko p) t -> p ko t", p=P)
            for k in range(NKO):
                nc.sync.dma_start(XT[:, k, :], xr[:, k, :])
            for wsb, wd in ((WQT, wqT_d), (WKT, wkT_d), (WVT, wvT_d)):
                wr = wd.rearrange("(ko p) n -> p ko n", p=P)
                for k in range(NKO):
                    nc.sync.dma_start(wsb[:, k, :], wr[:, k, :])
            wor = woT_d.rearrange("(jo p) n -> p jo n", p=P)
            for j in range(DG // P):
                nc.sync.dma_start(WOT[:, j, :], wor[:, j, :])

            # v_aug ones column
            nc.gpsimd.memset(VA[:, :, :, DH], 1.0)

            # ---- projections ----
            # qT/kT: out[dg, t] with dg on partitions (4 tiles of 128)
            for wsb, dst in ((WQT, QT), (WKT, KT)):
                for dg in range(DG // P):
                    for c in range(NCH):
                        ps = psp.tile([P, NC_CHUNK], F32, tag="s")
                        for k in range(NKO):
                            _mm(
                                nc, ps,
                                lhsT=wsb[:, k, ts(dg, P)],
                                rhs=XT[:, k, ds(c * NC_CHUNK, NC_CHUNK)],
                                start=(k == 0), stop=(k == NKO - 1),
                            )
                        nc.vector.tensor_copy(dst[:, dg, ds(c * NC_CHUNK, NC_CHUNK)], ps)
            # v: natural [t, dg] layout, written per-head into VA
            for tt in range(NT):
                ps = psp.tile([P, DG], F32, tag="s")
                for k in range(NKO):
                    nc.tensor.matmul(
                        ps,
                        lhsT=XT[:, k, ts(tt, P)],
                        rhs=WVT[:, k, :],
                        start=(k == 0), stop=(k == NKO - 1),
                    )
                nc.vector.tensor_copy(
                    VA[:, tt, :, 0:DH],
                    ps.rearrange("p (h d) -> p h d", h=NH),
                )

            # ---- attention (per head, per tq chunk) ----
            for h in range(NH):
                pt, po = h // 2, (h % 2) * DH
                qTh = QT[po:po + DH, pt]
                kTh = KT[po:po + DH, pt]
                stg = None
                if h % 2 == 1:
                    stg = stgp.tile([DH, T], BF16, tag="stg")
                for c in range(NCH):
                    av = avp.tile([P, NC_CHUNK], F32, tag="av")
                    jmax = (c + 1) * NC_CHUNK // P - 1
                    for j in range(jmax + 1):
                        lo = max(c * NC_CHUNK, j * P)
                        w = (c + 1) * NC_CHUNK - lo
                        diag = j * P >= c * NC_CHUNK
                        ps = psp.tile([P, NC_CHUNK], F32, tag="s")
                        if diag:
                            # first <=512 chunk holds the 128 masked columns
                            w0 = min(MM_N, w)
                            nc.tensor.matmul(
                                ps[:, 0:w0],
                                lhsT=kTh[:, ts(j, P)],
                                rhs=qTh[:, ds(lo, w0)],
                                start=True, stop=False,
                            )
                            nc.tensor.matmul(
                                ps[:, 0:P],
                                lhsT=idn, rhs=msk,
                                start=False, stop=True,
                            )
                            if w > w0:
                                _mm(
                                    nc, ps[:, ds(w0, w - w0)],
                                    lhsT=kTh[:, ts(j, P)],
                                    rhs=qTh[:, ds(lo + w0, w - w0)],
                                    start=True, stop=True,
                                )
                        else:
                            _mm(
                                nc, ps[:, :w],
                                lhsT=kTh[:, ts(j, P)],
                                rhs=qTh[:, ds(lo, w)],
                                start=True, stop=True,
                            )
                        et = work.tile([P, NC_CHUNK], BF16, tag="et")
                        nc.scalar.activation(
                            et[:, :w], ps[:, :w],
                            mybir.ActivationFunctionType.Exp,
                            scale=0.125,
                        )
                        # AV accumulate, per psum bank: bank b of this chunk
                        # ([512b, 512b+512)) has its last write at tile
                        # j == 8c + 4b + 3, which carries stop=True.
                        s0 = lo - c * NC_CHUNK
                        for b in range(NC_CHUNK // MM_N):
                            blo, bhi = b * MM_N, (b + 1) * MM_N
                            plo, phi = max(s0, blo), min(s0 + w, bhi)
                            if plo >= phi:
                                continue
                            nc.tensor.matmul(
                                av[0:DH + 1, ds(plo, phi - plo)],
                                lhsT=VA[:, j, h, :],
                                rhs=et[:, ds(plo - s0, phi - plo)],
                                start=(j == 0),
                                stop=(j == 8 * c + 4 * b + 3),
                            )
                    # normalize: rows 0..63 scaled by 1/row64, per column
                    rec = work.tile([1, NC_CHUNK], F32, tag="rec")
                    nc.vector.reciprocal(rec, av[DH:DH + 1, :])
                    recb = work.tile([1, NC_CHUNK], BF16, tag="recb")
                    nc.vector.tensor_copy(recb, rec)
                    bc = psp.tile([DH, NC_CHUNK], F32, tag="s")
                    _mm(nc, bc, lhsT=ones64, rhs=recb, start=True, stop=True)
                    bcb = work.tile([DH, NC_CHUNK], BF16, tag="bcb")
                    nc.vector.tensor_copy(bcb, bc)
                    if h % 2 == 0:
                        nc.vector.tensor_mul(
                            OGT[0:DH, pt, ds(c * NC_CHUNK, NC_CHUNK)],
                            av[0:DH, :], bcb,
                        )
                    else:
                        nc.vector.tensor_mul(
                            stg[:, ds(c * NC_CHUNK, NC_CHUNK)],
                            av[0:DH, :], bcb,
                        )
                if h % 2 == 1:
                    # partition shift 0-63 -> 64-127 via sbuf-to-sbuf DMA
                    nc.sync.dma_start(OGT[DH:P, pt, :], stg[:, :])

            # ---- o_proj partial: y[t, o] = sum_j ogT[j, t] * woT[j, o] ----
            for tt in range(NT):
                ps = psp.tile([P, D], F32, tag="s")
                for jt in range(DG // P):
                    _mm(
                        nc, ps,
                        lhsT=OGT[:, jt, ts(tt, P)],
                        rhs=WOT[:, jt, :],
                        start=(jt == 0), stop=(jt == DG // P - 1),
                    )
                ysb = work.tile([P, D], F32, tag="ysb")
                nc.vector.tensor_copy(ysb, ps)
                nc.sync.dma_start(y_d[ts(tt, P), :], ysb)

    nc.compile()
    return nc


def _get_nc():
    global _CACHED
    if _CACHED is None:
        _CACHED = _build_kernel()
    return _CACHED


def _shard_inputs(x, wq, wk, wv, wo):
    bf = ml_dtypes.bfloat16
    in_maps = []
    for core in range(8):
        b, g = divmod(core, 2)
        gs = slice(g * DG, (g + 1) * DG)
        in_maps.append({
            "xT": np.ascontiguousarray(x[b].T).astype(bf),
            "wqT": np.ascontiguousarray(wq[gs, :].T).astype(bf),
            "wkT": np.ascontiguousarray(wk[gs, :].T).astype(bf),
            "wvT": np.ascontiguousarray(wv[gs, :].T).astype(bf),
            "woT": np.ascontiguousarray(wo[:, gs].T).astype(bf),
        })
    return in_maps


def kernel(x, wq, wk, wv, wo, _trace=False, _trace_cores=None):
    x = np.asarray(x, dtype=np.float32)
    wq = np.asarray(wq, dtype=np.float32)
    wk = np.asarray(wk, dtype=np.float32)
    wv = np.asarray(wv, dtype=np.float32)
    wo = np.asarray(wo, dtype=np.float32)

    nc = _get_nc()
    in_maps = _shard_inputs(x, wq, wk, wv, wo)
    res = run_bass_kernel_spmd(
        nc, in_maps, core_ids=list(range(8)),
        trace=_trace,
        **({"trace_cores": _trace_cores} if _trace_cores else {}),
    )
    B = x.shape[0]
    y = np.zeros((B, T, D), dtype=np.float32)
    for core in range(8):
        b = core // 2
        y[b] += res.results[core]["y"]
    if _trace:
        return y, res
    return y


# revision 19
# speedup vs baseline: 1.2000x; 1.2000x over previous
"""Causal self-attention kernel for Trainium2, sharded over 8 NeuronCores.

Problem: B=4, T=2048, DIM=1024, H=16 heads, head_dim=64, fp32 I/O.

Sharding: (batch, head-group) pairs -> 8 shards. Core c handles batch
b = c//2 and head group g = c%2 (8 heads each). Each core computes its
q/k/v projections for its head slice, causal flash-style attention, and
a partial o_proj against its head-slice of wo. The host sums the two
partial o_proj outputs per batch (the "all-reduce") while gathering.

Layout strategy (per core):
  - Host pre-transposes x and the weight slices so the contraction dim
    (model dim) lands on SBUF partitions, and casts them to bf16.
  - Scores are computed TRANSPOSED: sT[tk, tq] = k @ q^T, so softmax'd
    probabilities come out with tk on partitions -- exactly the layout
    the attn@v matmul needs as its moving operand (lhsT = v).
  - Softmax skips max-subtraction (scores are O(1) by construction:
    q,k ~ N(0,1), dot/8), exp runs on the scalar engine straight out of
    PSUM, and the denominator is obtained for free by augmenting v with
    a ones column.
  - Causal masking inside diagonal 128-tiles is applied by one extra
    accumulating matmul (identity x (-1e9 strictly-lower-tri mask)).
"""

import numpy as np
import ml_dtypes

import concourse.bass as bass
import concourse.bacc as bacc
import concourse.mybir as mybir
import concourse.tile as tile
from concourse.bass import ds, ts
from concourse.bass_utils import run_bass_kernel_spmd
from concourse.masks import make_identity

BF16 = mybir.dt.bfloat16
F32 = mybir.dt.float32

T = 2048
D = 1024
DG = 512          # head-group width (8 heads x 64)
NH = 8            # heads per core
DH = 64
P = 128
NT = T // P       # 16 t-tiles
NKO = D // P      # 8 contraction tiles for projections
NC_CHUNK = 1024   # tq chunk width for attention
NCH = T // NC_CHUNK  # 2 chunks

_CACHED = None  # (nc, input names) -- build/trace once per process

MM_N = 512  # max moving free-dim per matmul instruction


def _mm(nc, out, lhsT, rhs, start, stop, out_off=0):
    """matmul out = lhsT.T @ rhs, sliced so no piece crosses a PSUM bank
    boundary. out_off is the column offset of `out` within its psum tile."""
    n = rhs.shape[-1]
    o = 0
    while o < n:
        w = min(n - o, MM_N - ((out_off + o) % MM_N))
        nc.tensor.matmul(
            out[:, ds(o, w)], lhsT=lhsT, rhs=rhs[:, ds(o, w)],
            start=start, stop=stop,
        )
        o += w


def _build_kernel():
    nc = bacc.Bacc("TRN2", target_bir_lowering=False, debug=False)

    xT_d = nc.dram_tensor("xT", [D, T], BF16, kind="ExternalInput").ap()
    wqT_d = nc.dram_tensor("wqT", [D, DG], BF16, kind="ExternalInput").ap()
    wkT_d = nc.dram_tensor("wkT", [D, DG], BF16, kind="ExternalInput").ap()
    wvT_d = nc.dram_tensor("wvT", [D, DG], BF16, kind="ExternalInput").ap()
    woT_d = nc.dram_tensor("woT", [DG, D], BF16, kind="ExternalInput").ap()
    y_d = nc.dram_tensor("y", [T, D], F32, kind="ExternalOutput").ap()

    with tile.TileContext(nc) as tc:
        with (
            tc.tile_pool(name="const", bufs=1) as const,
            tc.tile_pool(name="sb", bufs=1) as sb,
            tc.tile_pool(name="work", bufs=3) as work,
            tc.tile_pool(name="stgp", bufs=2) as stgp,
            tc.tile_pool(name="ps", bufs=2, space="PSUM") as psp,
            tc.tile_pool(name="av", bufs=2, space="PSUM") as avp,
        ):
            # ---- constants ----
            idn = const.tile([P, P], BF16, tag="idn")
            make_identity(nc, idn)
            msk = const.tile([P, P], BF16, tag="msk")
            # msk[tk, tq] = 0 where tq >= tk else -1e9  (strictly-lower = masked)
            nc.gpsimd.memset(msk, 0.0)
            nc.gpsimd.affine_select(
                out=msk, in_=msk,
                compare_op=mybir.AluOpType.is_ge,
                fill=-1e9, base=0,
                pattern=[[1, P]], channel_multiplier=-1,
            )
            ones64 = const.tile([1, DH], BF16, tag="ones64")
            nc.gpsimd.memset(ones64, 1.0)

            # ---- persistent SBUF tensors ----
            XT = sb.tile([P, NKO, T], BF16, tag="XT")
            WQT = sb.tile([P, NKO, DG], BF16, tag="WQT")
            WKT = sb.tile([P, NKO, DG], BF16, tag="WKT")
            WVT = sb.tile([P, NKO, DG], BF16, tag="WVT")
            WOT = sb.tile([P, DG // P, D], BF16, tag="WOT")
            QT = sb.tile([P, DG // P, T], BF16, tag="QT")
            KT = sb.tile([P, DG // P, T], BF16, tag="KT")
            VA = sb.tile([P, NT, NH, DH + 1], BF16, tag="VA")
            OGT = sb.tile([P, DG // P, T], BF16, tag="OGT")

            # ---- input DMAs (chunked across queues) ----
            xr = xT_d.rearrange("(ko p) t -> p ko t", p=P)
            for k in range(NKO):
                nc.sync.dma_start(XT[:, k, :], xr[:, k, :])
            for wsb, wd in ((WQT, wqT_d), (WKT, wkT_d), (WVT, wvT_d)):
                wr = wd.rearrange("(ko p) n -> p ko n", p=P)
                for k in range(NKO):
                    nc.sync.dma_start(wsb[:, k, :], wr[:, k, :])
            wor = woT_d.rearrange("(jo p) n -> p jo n", p=P)
            for j in range(DG // P):
                nc.sync.dma_start(WOT[:, j, :], wor[:, j, :])

            # v_aug ones column
            nc.gpsimd.memset(VA[:, :, :, DH], 1.0)

            # ---- projections ----
            # qT/kT: out[dg, t] with dg on partitions (4 tiles of 128)
            for wsb, dst in ((WQT, QT), (WKT, KT)):
                for dg in range(DG // P):
                    for c in range(NCH):
                        ps = psp.tile([P, NC_CHUNK], F32, tag="s")
                        for k in range(NKO):
                            _mm(
                                nc, ps,
                                lhsT=wsb[:, k, ts(dg, P)],
                                rhs=XT[:, k, ds(c * NC_CHUNK, NC_CHUNK)],
                                start=(k == 0), stop=(k == NKO - 1),
                            )
                        nc.vector.tensor_copy(dst[:, dg, ds(c * NC_CHUNK, NC_CHUNK)], ps)
            # v: natural [t, dg] layout, written per-head into VA
            for tt in range(NT):
                ps = psp.tile([P, DG], F32, tag="s")
                for k in range(NKO):
                    nc.tensor.matmul(
                        ps,
                        lhsT=XT[:, k, ts(tt, P)],
                        rhs=WVT[:, k, :],
                        start=(k == 0), stop=(k == NKO - 1),
                    )
                nc.vector.tensor_copy(
                    VA[:, tt, :, 0:DH],
                    ps.rearrange("p (h d) -> p h d", h=NH),
                )

            # ---- attention (head pairs interleaved, per tq chunk) ----
            # Paired heads live at partitions 0-63 / 64-127 of the same
            # QT/KT p-tile, so their score matmuls use disjoint PE row
            # groups (concurrent) and the pair keeps the PE fed while the
            # scalar engine runs exp for the other head.
            def attn_head_tile(h, po, pt, av, c, j, jmax):
                qTh = QT[po:po + DH, pt]
                kTh = KT[po:po + DH, pt]
                lo = max(c * NC_CHUNK, j * P)
                w = (c + 1) * NC_CHUNK - lo
                diag = j * P >= c * NC_CHUNK
                ps = psp.tile([P, NC_CHUNK], F32, tag="s")
                if diag:
                    # first <=512 chunk holds the 128 masked columns
                    w0 = min(MM_N, w)
                    nc.tensor.matmul(
                        ps[:, 0:w0],
                        lhsT=kTh[:, ts(j, P)],
                        rhs=qTh[:, ds(lo, w0)],
                        start=True, stop=False,
                    )
                    nc.tensor.matmul(
                        ps[:, 0:P],
                        lhsT=idn, rhs=msk,
                        start=False, stop=True,
                    )
                    if w > w0:
                        _mm(
                            nc, ps[:, ds(w0, w - w0)],
                            lhsT=kTh[:, ts(j, P)],
                            rhs=qTh[:, ds(lo + w0, w - w0)],
                            start=True, stop=True,
                        )
                else:
                    _mm(
                        nc, ps[:, :w],
                        lhsT=kTh[:, ts(j, P)],
                        rhs=qTh[:, ds(lo, w)],
                        start=True, stop=True,
                    )
                et = work.tile([P, NC_CHUNK], BF16, tag="et")
                nc.scalar.activation(
                    et[:, :w], ps[:, :w],
                    mybir.ActivationFunctionType.Exp,
                    scale=0.125,
                )
                # AV accumulate, per psum bank: bank b of this chunk
                # ([512b, 512b+512)) has its last write at tile
                # j == 8c + 4b + 3, which carries stop=True.
                s0 = lo - c * NC_CHUNK
                for b in range(NC_CHUNK // MM_N):
                    blo, bhi = b * MM_N, (b + 1) * MM_N
                    plo, phi = max(s0, blo), min(s0 + w, bhi)
                    if plo >= phi:
                        continue
                    nc.tensor.matmul(
                        av[0:DH + 1, ds(plo, phi - plo)],
                        lhsT=VA[:, j, h, :],
                        rhs=et[:, ds(plo - s0, phi - plo)],
                        start=(j == 0),
                        stop=(j == 8 * c + 4 * b + 3),
                    )

            def attn_normalize(av, dst):
                # dst: [DH, NC_CHUNK] slice; scale av rows 0..63 by 1/row64.
                # 1/d as exp(-ln d) on ScalarE: d is a positive softmax
                # denominator and the product feeds a bf16 multiply, so ACT
                # table accuracy is plenty; keeps the slow DVE RECIPROCAL
                # (6.5us for a 1-partition row) off the critical path.
                rec = work.tile([1, NC_CHUNK], F32, tag="rec")
                nc.scalar.activation(
                    rec, av[DH:DH + 1, :], mybir.ActivationFunctionType.Ln,
                )
                recb = work.tile([1, NC_CHUNK], BF16, tag="recb")
                nc.scalar.activation(
                    recb, rec, mybir.ActivationFunctionType.Exp, scale=-1.0,
                )
                bc = psp.tile([DH, NC_CHUNK], F32, tag="s")
                _mm(nc, bc, lhsT=ones64, rhs=recb, start=True, stop=True)
                bcb = work.tile([DH, NC_CHUNK], BF16, tag="bcb")
                nc.vector.tensor_copy(bcb, bc)
                nc.vector.tensor_mul(dst, av[0:DH, :], bcb)

            for hp in range(NH // 2):
                hA, hB = 2 * hp, 2 * hp + 1
                stg = stgp.tile([DH, T], BF16, tag="stg")
                for c in range(NCH):
                    avA = avp.tile([P, NC_CHUNK], F32, tag="av")
                    avB = avp.tile([P, NC_CHUNK], F32, tag="av")
                    jmax = (c + 1) * NC_CHUNK // P - 1
                    for j in range(jmax + 1):
                        attn_head_tile(hA, 0, hp, avA, c, j, jmax)
                        attn_head_tile(hB, DH, hp, avB, c, j, jmax)
                    attn_normalize(avA, OGT[0:DH, hp, ds(c * NC_CHUNK, NC_CHUNK)])
                    attn_normalize(avB, stg[:, ds(c * NC_CHUNK, NC_CHUNK)])
                # partition shift 0-63 -> 64-127 via sbuf-to-sbuf DMA
                nc.sync.dma_start(OGT[DH:P, hp, :], stg[:, :])

            # ---- o_proj partial: y[t, o] = sum_j ogT[j, t] * woT[j, o] ----
            for tt in range(NT):
                ps = psp.tile([P, D], F32, tag="s")
                for jt in range(DG // P):
                    _mm(
                        nc, ps,
                        lhsT=OGT[:, jt, ts(tt, P)],
                        rhs=WOT[:, jt, :],
                        start=(jt == 0), stop=(jt == DG // P - 1),
                    )
                ysb = work.tile([P, D], F32, tag="ysb")
                nc.vector.tensor_copy(ysb, ps)
                nc.sync.dma_start(y_d[ts(tt, P), :], ysb)

    nc.compile()
    return nc


def _get_nc():
    global _CACHED
    if _CACHED is None:
        _CACHED = _build_kernel()
    return _CACHED


def _shard_inputs(x, wq, wk, wv, wo):
    bf = ml_dtypes.bfloat16
    in_maps = []
    for core in range(8):
        b, g = divmod(core, 2)
        gs = slice(g * DG, (g + 1) * DG)
        in_maps.append({
            "xT": np.ascontiguousarray(x[b].T).astype(bf),
            "wqT": np.ascontiguousarray(wq[gs, :].T).astype(bf),
            "wkT": np.ascontiguousarray(wk[gs, :].T).astype(bf),
            "wvT": np.ascontiguousarray(wv[gs, :].T).astype(bf),
            "woT": np.ascontiguousarray(wo[:, gs].T).astype(bf),
        })
    return in_maps


def kernel(x, wq, wk, wv, wo, _trace=False, _trace_cores=None):
    x = np.asarray(x, dtype=np.float32)
    wq = np.asarray(wq, dtype=np.float32)
    wk = np.asarray(wk, dtype=np.float32)
    wv = np.asarray(wv, dtype=np.float32)
    wo = np.asarray(wo, dtype=np.float32)

    nc = _get_nc()
    in_maps = _shard_inputs(x, wq, wk, wv, wo)
    res = run_bass_kernel_spmd(
        nc, in_maps, core_ids=list(range(8)),
        trace=_trace,
        **({"trace_cores": _trace_cores} if _trace_cores else {}),
    )
    B = x.shape[0]
    y = np.zeros((B, T, D), dtype=np.float32)
    for core in range(8):
        b = core // 2
        y[b] += res.results[core]["y"]
    if _trace:
        return y, res
    return y


# revision 25
# speedup vs baseline: 1.3203x; 1.1002x over previous
"""Causal self-attention kernel for Trainium2, sharded over 8 NeuronCores.

Problem: B=4, T=2048, DIM=1024, H=16 heads, head_dim=64, fp32 I/O.

Sharding: (batch, head-group) pairs -> 8 shards. Core c handles batch
b = c//2 and head group g = c%2 (8 heads each). Each core computes its
q/k/v projections for its head slice, causal flash-style attention, and
a partial o_proj against its head-slice of wo. The host sums the two
partial o_proj outputs per batch (the "all-reduce") while gathering.

Layout strategy (per core):
  - Host pre-transposes x and the weight slices so the contraction dim
    (model dim) lands on SBUF partitions, and casts them to bf16.
  - Scores are computed TRANSPOSED: sT[tk, tq] = k @ q^T, so softmax'd
    probabilities come out with tk on partitions -- exactly the layout
    the attn@v matmul needs as its moving operand (lhsT = v).
  - Softmax skips max-subtraction (scores are O(1) by construction:
    q,k ~ N(0,1), dot/8), exp runs on the scalar engine straight out of
    PSUM, and the denominator is obtained for free by augmenting v with
    a ones column.
  - Causal masking inside diagonal 128-tiles is applied by one extra
    accumulating matmul (identity x (-1e9 strictly-lower-tri mask)).
"""

import numpy as np
import ml_dtypes

import concourse.bass as bass
import concourse.bacc as bacc
import concourse.mybir as mybir
import concourse.tile as tile
from concourse.bass import ds, ts
from concourse.bass_utils import run_bass_kernel_spmd

BF16 = mybir.dt.bfloat16
F32 = mybir.dt.float32

T = 2048
D = 1024
DG = 512          # head-group width (8 heads x 64)
NH = 8            # heads per core
DH = 64
P = 128
NT = T // P       # 16 t-tiles
NKO = D // P      # 8 contraction tiles for projections
NC_CHUNK = 1024   # tq chunk width for attention
NCH = T // NC_CHUNK  # 2 chunks

_CACHED = None  # (nc, input names) -- build/trace once per process

MM_N = 512  # max moving free-dim per matmul instruction


def _mm(nc, out, lhsT, rhs, start, stop, out_off=0):
    """matmul out = lhsT.T @ rhs, sliced so no piece crosses a PSUM bank
    boundary. out_off is the column offset of `out` within its psum tile."""
    n = rhs.shape[-1]
    o = 0
    while o < n:
        w = min(n - o, MM_N - ((out_off + o) % MM_N))
        nc.tensor.matmul(
            out[:, ds(o, w)], lhsT=lhsT, rhs=rhs[:, ds(o, w)],
            start=start, stop=stop,
        )
        o += w


def _build_kernel():
    nc = bacc.Bacc("TRN2", target_bir_lowering=False, debug=False)

    xT_d = nc.dram_tensor("xT", [D, T], BF16, kind="ExternalInput").ap()
    wqT_d = nc.dram_tensor("wqT", [D, DG], BF16, kind="ExternalInput").ap()
    wkT_d = nc.dram_tensor("wkT", [D, DG], BF16, kind="ExternalInput").ap()
    wvT_d = nc.dram_tensor("wvT", [D, DG], BF16, kind="ExternalInput").ap()
    woT_d = nc.dram_tensor("woT", [DG, D], BF16, kind="ExternalInput").ap()
    y_d = nc.dram_tensor("y", [T, D], F32, kind="ExternalOutput").ap()

    with tile.TileContext(nc) as tc:
        with (
            tc.tile_pool(name="const", bufs=1) as const,
            tc.tile_pool(name="sb", bufs=1) as sb,
            tc.tile_pool(name="work", bufs=4) as work,
            tc.tile_pool(name="stgp", bufs=2) as stgp,
            tc.tile_pool(name="ps", bufs=2, space="PSUM") as psp,
            tc.tile_pool(name="av", bufs=2, space="PSUM") as avp,
        ):
            # ---- constants ----
            ones64 = const.tile([1, DH], BF16, tag="ones64")
            nc.gpsimd.memset(ones64, 1.0)
            # multiplicative causal mask for diag tiles: 1 where tq >= tk
            mskb = const.tile([P, P], BF16, tag="mskb")
            nc.gpsimd.memset(mskb, 1.0)
            nc.gpsimd.affine_select(
                out=mskb, in_=mskb,
                compare_op=mybir.AluOpType.is_ge,
                fill=0.0, base=0,
                pattern=[[1, P]], channel_multiplier=-1,
            )

            # ---- persistent SBUF tensors ----
            XT = sb.tile([P, NKO, T], BF16, tag="XT")
            WQT = sb.tile([P, NKO, DG], BF16, tag="WQT")
            WKT = sb.tile([P, NKO, DG], BF16, tag="WKT")
            WVT = sb.tile([P, NKO, DG], BF16, tag="WVT")
            WOT = sb.tile([P, DG // P, D], BF16, tag="WOT")
            QT = sb.tile([P, DG // P, T], BF16, tag="QT")
            KT = sb.tile([P, DG // P, T], BF16, tag="KT")
            VA = sb.tile([P, NT, NH, DH + 1], BF16, tag="VA")
            OGT = sb.tile([P, DG // P, T], BF16, tag="OGT")

            # ---- input DMAs (chunked across queues) ----
            xr = xT_d.rearrange("(ko p) t -> p ko t", p=P)
            for k in range(NKO):
                nc.sync.dma_start(XT[:, k, :], xr[:, k, :])
            for wsb, wd in ((WQT, wqT_d), (WKT, wkT_d), (WVT, wvT_d)):
                wr = wd.rearrange("(ko p) n -> p ko n", p=P)
                for k in range(NKO):
                    nc.sync.dma_start(wsb[:, k, :], wr[:, k, :])
            wor = woT_d.rearrange("(jo p) n -> p jo n", p=P)
            for j in range(DG // P):
                nc.sync.dma_start(WOT[:, j, :], wor[:, j, :])

            # v_aug ones column
            nc.gpsimd.memset(VA[:, :, :, DH], 1.0)

            # ---- projections ----
            # qT/kT: out[dg, t] with dg on partitions (4 tiles of 128)
            for wsb, dst in ((WQT, QT), (WKT, KT)):
                for dg in range(DG // P):
                    for c in range(NCH):
                        ps = psp.tile([P, NC_CHUNK], F32, tag="s")
                        for k in range(NKO):
                            _mm(
                                nc, ps,
                                lhsT=wsb[:, k, ts(dg, P)],
                                rhs=XT[:, k, ds(c * NC_CHUNK, NC_CHUNK)],
                                start=(k == 0), stop=(k == NKO - 1),
                            )
                        nc.vector.tensor_copy(dst[:, dg, ds(c * NC_CHUNK, NC_CHUNK)], ps)
            # v: natural [t, dg] layout, written per-head into VA
            for tt in range(NT):
                ps = psp.tile([P, DG], F32, tag="s")
                for k in range(NKO):
                    nc.tensor.matmul(
                        ps,
                        lhsT=XT[:, k, ts(tt, P)],
                        rhs=WVT[:, k, :],
                        start=(k == 0), stop=(k == NKO - 1),
                    )
                nc.vector.tensor_copy(
                    VA[:, tt, :, 0:DH],
                    ps.rearrange("p (h d) -> p h d", h=NH),
                )

            # ---- attention (head pairs interleaved, per tq chunk) ----
            # Paired heads live at partitions 0-63 / 64-127 of the same
            # QT/KT p-tile, so their score matmuls use disjoint PE row
            # groups (concurrent) and the pair keeps the PE fed while the
            # scalar engine runs exp for the other head.
            def attn_scores_exp(po, pt, c, j):
                """scores + exp for one head tile; returns the expT tile."""
                qTh = QT[po:po + DH, pt]
                kTh = KT[po:po + DH, pt]
                lo = max(c * NC_CHUNK, j * P)
                w = (c + 1) * NC_CHUNK - lo
                diag = j * P >= c * NC_CHUNK
                ps = psp.tile([P, NC_CHUNK], F32, tag="s")
                _mm(
                    nc, ps[:, :w],
                    lhsT=kTh[:, ts(j, P)],
                    rhs=qTh[:, ds(lo, w)],
                    start=True, stop=True,
                )
                et = work.tile([P, NC_CHUNK], BF16, tag="et")
                nc.scalar.activation(
                    et[:, :w], ps[:, :w],
                    mybir.ActivationFunctionType.Exp,
                    scale=0.125,
                )
                if diag:
                    # zero the lower-left of the diagonal 128-block (DVE is
                    # idle; keeps the mask off the busy PE)
                    nc.vector.tensor_mul(et[:, 0:P], et[:, 0:P], mskb)
                return et

            def attn_av(h, av, et, c, j):
                # AV accumulate, per psum bank: bank b of this chunk
                # ([512b, 512b+512)) has its last write at tile
                # j == 8c + 4b + 3, which carries stop=True.
                lo = max(c * NC_CHUNK, j * P)
                w = (c + 1) * NC_CHUNK - lo
                s0 = lo - c * NC_CHUNK
                for b in range(NC_CHUNK // MM_N):
                    blo, bhi = b * MM_N, (b + 1) * MM_N
                    plo, phi = max(s0, blo), min(s0 + w, bhi)
                    if plo >= phi:
                        continue
                    nc.tensor.matmul(
                        av[0:DH + 1, ds(plo, phi - plo)],
                        lhsT=VA[:, j, h, :],
                        rhs=et[:, ds(plo - s0, phi - plo)],
                        start=(j == 0),
                        stop=(j == 8 * c + 4 * b + 3),
                    )

            def attn_normalize(av, dst):
                # dst: [DH, NC_CHUNK] slice; scale av rows 0..63 by 1/row64.
                # 1/d as exp(-ln d) on ScalarE: d is a positive softmax
                # denominator and the product feeds a bf16 multiply, so ACT
                # table accuracy is plenty; keeps the slow DVE RECIPROCAL
                # (6.5us for a 1-partition row) off the critical path.
                rec = work.tile([1, NC_CHUNK], F32, tag="rec")
                nc.scalar.activation(
                    rec, av[DH:DH + 1, :], mybir.ActivationFunctionType.Ln,
                )
                recb = work.tile([1, NC_CHUNK], BF16, tag="recb")
                nc.scalar.activation(
                    recb, rec, mybir.ActivationFunctionType.Exp, scale=-1.0,
                )
                bc = psp.tile([DH, NC_CHUNK], F32, tag="s")
                _mm(nc, bc, lhsT=ones64, rhs=recb, start=True, stop=True)
                bcb = work.tile([DH, NC_CHUNK], BF16, tag="bcb")
                nc.vector.tensor_copy(bcb, bc)
                nc.vector.tensor_mul(dst, av[0:DH, :], bcb)

            for hp in range(NH // 2):
                hA, hB = 2 * hp, 2 * hp + 1
                stg = stgp.tile([DH, T], BF16, tag="stg")
                for c in range(NCH):
                    avA = avp.tile([P, NC_CHUNK], F32, tag="av")
                    avB = avp.tile([P, NC_CHUNK], F32, tag="av")
                    jmax = (c + 1) * NC_CHUNK // P - 1
                    for j in range(jmax + 1):
                        etA = attn_scores_exp(0, hp, c, j)
                        etB = attn_scores_exp(DH, hp, c, j)
                        attn_av(hA, avA, etA, c, j)
                        attn_av(hB, avB, etB, c, j)
                    attn_normalize(avA, OGT[0:DH, hp, ds(c * NC_CHUNK, NC_CHUNK)])
                    attn_normalize(avB, stg[:, ds(c * NC_CHUNK, NC_CHUNK)])
                # partition shift 0-63 -> 64-127 via sbuf-to-sbuf DMA
                nc.sync.dma_start(OGT[DH:P, hp, :], stg[:, :])

            # ---- o_proj partial: y[t, o] = sum_j ogT[j, t] * woT[j, o] ----
            for tt in range(NT):
                ps = psp.tile([P, D], F32, tag="s")
                for jt in range(DG // P):
                    _mm(
                        nc, ps,
                        lhsT=OGT[:, jt, ts(tt, P)],
                        rhs=WOT[:, jt, :],
                        start=(jt == 0), stop=(jt == DG // P - 1),
                    )
                ysb = work.tile([P, D], F32, tag="ysb")
                nc.vector.tensor_copy(ysb, ps)
                nc.sync.dma_start(y_d[ts(tt, P), :], ysb)

    nc.compile()
    return nc


def _get_nc():
    global _CACHED
    if _CACHED is None:
        _CACHED = _build_kernel()
    return _CACHED


def _shard_inputs(x, wq, wk, wv, wo):
    bf = ml_dtypes.bfloat16
    in_maps = []
    for core in range(8):
        b, g = divmod(core, 2)
        gs = slice(g * DG, (g + 1) * DG)
        in_maps.append({
            "xT": np.ascontiguousarray(x[b].T).astype(bf),
            "wqT": np.ascontiguousarray(wq[gs, :].T).astype(bf),
            "wkT": np.ascontiguousarray(wk[gs, :].T).astype(bf),
            "wvT": np.ascontiguousarray(wv[gs, :].T).astype(bf),
            "woT": np.ascontiguousarray(wo[:, gs].T).astype(bf),
        })
    return in_maps


def kernel(x, wq, wk, wv, wo, _trace=False, _trace_cores=None):
    x = np.asarray(x, dtype=np.float32)
    wq = np.asarray(wq, dtype=np.float32)
    wk = np.asarray(wk, dtype=np.float32)
    wv = np.asarray(wv, dtype=np.float32)
    wo = np.asarray(wo, dtype=np.float32)

    nc = _get_nc()
    in_maps = _shard_inputs(x, wq, wk, wv, wo)
    res = run_bass_kernel_spmd(
        nc, in_maps, core_ids=list(range(8)),
        trace=_trace,
        **({"trace_cores": _trace_cores} if _trace_cores else {}),
    )
    B = x.shape[0]
    y = np.zeros((B, T, D), dtype=np.float32)
    for core in range(8):
        b = core // 2
        y[b] += res.results[core]["y"]
    if _trace:
        return y, res
    return y


# revision 28
# speedup vs baseline: 1.3476x; 1.0207x over previous
"""Causal self-attention kernel for Trainium2, sharded over 8 NeuronCores.

Problem: B=4, T=2048, DIM=1024, H=16 heads, head_dim=64, fp32 I/O.

Sharding: (batch, head-group) pairs -> 8 shards. Core c handles batch
b = c//2 and head group g = c%2 (8 heads each). Each core computes its
q/k/v projections for its head slice, causal flash-style attention, and
a partial o_proj against its head-slice of wo. The host sums the two
partial o_proj outputs per batch (the "all-reduce") while gathering.

Layout strategy (per core):
  - Host pre-transposes x and the weight slices so the contraction dim
    (model dim) lands on SBUF partitions, and casts them to bf16.
  - Scores are computed TRANSPOSED: sT[tk, tq] = k @ q^T, so softmax'd
    probabilities come out with tk on partitions -- exactly the layout
    the attn@v matmul needs as its moving operand (lhsT = v).
  - Softmax skips max-subtraction (scores are O(1) by construction:
    q,k ~ N(0,1), dot/8), exp runs on the scalar engine straight out of
    PSUM, and the denominator is obtained for free by augmenting v with
    a ones column.
  - Causal masking inside diagonal 128-tiles is applied by one extra
    accumulating matmul (identity x (-1e9 strictly-lower-tri mask)).
"""

import numpy as np
import ml_dtypes

import concourse.bass as bass
import concourse.bacc as bacc
import concourse.mybir as mybir
import concourse.tile as tile
from concourse.bass import ds, ts
from concourse.bass_utils import run_bass_kernel_spmd

BF16 = mybir.dt.bfloat16
F32 = mybir.dt.float32

T = 2048
D = 1024
DG = 512          # head-group width (8 heads x 64)
NH = 8            # heads per core
DH = 64
P = 128
NT = T // P       # 16 t-tiles
NKO = D // P      # 8 contraction tiles for projections
NC_CHUNK = 1024   # tq chunk width for attention
NCH = T // NC_CHUNK  # 2 chunks

_CACHED = None  # (nc, input names) -- build/trace once per process

MM_N = 512  # max moving free-dim per matmul instruction


def _mm(nc, out, lhsT, rhs, start, stop, out_off=0):
    """matmul out = lhsT.T @ rhs, sliced so no piece crosses a PSUM bank
    boundary. out_off is the column offset of `out` within its psum tile."""
    n = rhs.shape[-1]
    o = 0
    while o < n:
        w = min(n - o, MM_N - ((out_off + o) % MM_N))
        nc.tensor.matmul(
            out[:, ds(o, w)], lhsT=lhsT, rhs=rhs[:, ds(o, w)],
            start=start, stop=stop,
        )
        o += w


def _build_kernel():
    nc = bacc.Bacc("TRN2", target_bir_lowering=False, debug=False)

    xT_d = nc.dram_tensor("xT", [D, T], BF16, kind="ExternalInput").ap()
    wqT_d = nc.dram_tensor("wqT", [D, DG], BF16, kind="ExternalInput").ap()
    wkT_d = nc.dram_tensor("wkT", [D, DG], BF16, kind="ExternalInput").ap()
    wvT_d = nc.dram_tensor("wvT", [D, DG], BF16, kind="ExternalInput").ap()
    woT_d = nc.dram_tensor("woT", [DG, D], BF16, kind="ExternalInput").ap()
    y_d = nc.dram_tensor("y", [T, D], F32, kind="ExternalOutput").ap()

    with tile.TileContext(nc) as tc:
        with (
            tc.tile_pool(name="const", bufs=1) as const,
            tc.tile_pool(name="sb", bufs=1) as sb,
            tc.tile_pool(name="work", bufs=4) as work,
            tc.tile_pool(name="stgp", bufs=2) as stgp,
            tc.tile_pool(name="ps", bufs=2, space="PSUM") as psp,
            tc.tile_pool(name="av", bufs=2, space="PSUM") as avp,
        ):
            # ---- constants ----
            # multiplicative causal mask for diag tiles: 1 where tq >= tk
            mskb = const.tile([P, P], BF16, tag="mskb")
            nc.gpsimd.memset(mskb, 1.0)
            nc.gpsimd.affine_select(
                out=mskb, in_=mskb,
                compare_op=mybir.AluOpType.is_ge,
                fill=0.0, base=0,
                pattern=[[1, P]], channel_multiplier=-1,
            )

            # ---- persistent SBUF tensors ----
            XT = sb.tile([P, NKO, T], BF16, tag="XT")
            WQT = sb.tile([P, NKO, DG], BF16, tag="WQT")
            WKT = sb.tile([P, NKO, DG], BF16, tag="WKT")
            WVT = sb.tile([P, NKO, DG], BF16, tag="WVT")
            WOT = sb.tile([P, DG // P, D], BF16, tag="WOT")
            QT = sb.tile([P, DG // P, T], BF16, tag="QT")
            KT = sb.tile([P, DG // P, T], BF16, tag="KT")
            VA = sb.tile([P, NT, NH, DH + 1], BF16, tag="VA")
            OGT = sb.tile([P, DG // P, T], BF16, tag="OGT")

            # ---- input DMAs (chunked across queues) ----
            xr = xT_d.rearrange("(ko p) t -> p ko t", p=P)
            for k in range(NKO):
                nc.sync.dma_start(XT[:, k, :], xr[:, k, :])
            for wsb, wd in ((WQT, wqT_d), (WKT, wkT_d), (WVT, wvT_d)):
                wr = wd.rearrange("(ko p) n -> p ko n", p=P)
                for k in range(NKO):
                    nc.sync.dma_start(wsb[:, k, :], wr[:, k, :])
            wor = woT_d.rearrange("(jo p) n -> p jo n", p=P)
            for j in range(DG // P):
                nc.sync.dma_start(WOT[:, j, :], wor[:, j, :])

            # v_aug ones column
            nc.gpsimd.memset(VA[:, :, :, DH], 1.0)

            # ---- projections ----
            # qT/kT: out[dg, t] with dg on partitions (4 tiles of 128)
            for wsb, dst in ((WQT, QT), (WKT, KT)):
                for dg in range(DG // P):
                    for c in range(NCH):
                        ps = psp.tile([P, NC_CHUNK], F32, tag="s")
                        for k in range(NKO):
                            _mm(
                                nc, ps,
                                lhsT=wsb[:, k, ts(dg, P)],
                                rhs=XT[:, k, ds(c * NC_CHUNK, NC_CHUNK)],
                                start=(k == 0), stop=(k == NKO - 1),
                            )
                        nc.vector.tensor_copy(dst[:, dg, ds(c * NC_CHUNK, NC_CHUNK)], ps)
            # v: natural [t, dg] layout, written per-head into VA
            for tt in range(NT):
                ps = psp.tile([P, DG], F32, tag="s")
                for k in range(NKO):
                    nc.tensor.matmul(
                        ps,
                        lhsT=XT[:, k, ts(tt, P)],
                        rhs=WVT[:, k, :],
                        start=(k == 0), stop=(k == NKO - 1),
                    )
                nc.vector.tensor_copy(
                    VA[:, tt, :, 0:DH],
                    ps.rearrange("p (h d) -> p h d", h=NH),
                )

            # ---- attention (head pairs interleaved, per tq chunk) ----
            # Paired heads live at partitions 0-63 / 64-127 of the same
            # QT/KT p-tile, so their score matmuls use disjoint PE row
            # groups (concurrent) and the pair keeps the PE fed while the
            # scalar engine runs exp for the other head.
            def attn_scores_exp(po, pt, c, j):
                """scores + exp for one head tile; returns the expT tile."""
                qTh = QT[po:po + DH, pt]
                kTh = KT[po:po + DH, pt]
                lo = max(c * NC_CHUNK, j * P)
                w = (c + 1) * NC_CHUNK - lo
                diag = j * P >= c * NC_CHUNK
                ps = psp.tile([P, NC_CHUNK], F32, tag="s")
                _mm(
                    nc, ps[:, :w],
                    lhsT=kTh[:, ts(j, P)],
                    rhs=qTh[:, ds(lo, w)],
                    start=True, stop=True,
                )
                et = work.tile([P, NC_CHUNK], BF16, tag="et")
                nc.scalar.activation(
                    et[:, :w], ps[:, :w],
                    mybir.ActivationFunctionType.Exp,
                    scale=0.125,
                )
                if diag:
                    # zero the lower-left of the diagonal 128-block (DVE is
                    # idle; keeps the mask off the busy PE)
                    nc.vector.tensor_mul(et[:, 0:P], et[:, 0:P], mskb)
                return et

            def attn_av(h, av, et, c, j):
                # AV accumulate, per psum bank: bank b of this chunk
                # ([512b, 512b+512)) has its last write at tile
                # j == 8c + 4b + 3, which carries stop=True.
                lo = max(c * NC_CHUNK, j * P)
                w = (c + 1) * NC_CHUNK - lo
                s0 = lo - c * NC_CHUNK
                for b in range(NC_CHUNK // MM_N):
                    blo, bhi = b * MM_N, (b + 1) * MM_N
                    plo, phi = max(s0, blo), min(s0 + w, bhi)
                    if plo >= phi:
                        continue
                    nc.tensor.matmul(
                        av[0:DH + 1, ds(plo, phi - plo)],
                        lhsT=VA[:, j, h, :],
                        rhs=et[:, ds(plo - s0, phi - plo)],
                        start=(j == 0),
                        stop=(j == 8 * c + 4 * b + 3),
                    )

            def attn_normalize(av, dst):
                # dst: [DH, NC_CHUNK] slice; scale av rows 0..63 by 1/row64.
                # 1/d as exp(-ln d) on ScalarE: d is a positive softmax
                # denominator and the product feeds a bf16 multiply, so ACT
                # table accuracy is plenty; keeps the slow DVE RECIPROCAL
                # (6.5us for a 1-partition row) off the critical path.
                rec = work.tile([1, NC_CHUNK], F32, tag="rec")
                nc.scalar.activation(
                    rec, av[DH:DH + 1, :], mybir.ActivationFunctionType.Ln,
                )
                recb = work.tile([1, NC_CHUNK], BF16, tag="recb")
                nc.scalar.activation(
                    recb, rec, mybir.ActivationFunctionType.Exp, scale=-1.0,
                )
                # broadcast 1/d across partitions on the (idle) GPSIMD so
                # the PE stream rolls straight into the next chunk
                bcb = work.tile([DH, NC_CHUNK], BF16, tag="bcb")
                nc.gpsimd.partition_broadcast(bcb, recb)
                nc.vector.tensor_mul(dst, av[0:DH, :], bcb)

            for hp in range(NH // 2):
                hA, hB = 2 * hp, 2 * hp + 1
                stg = stgp.tile([DH, T], BF16, tag="stg")
                for c in range(NCH):
                    avA = avp.tile([P, NC_CHUNK], F32, tag="av")
                    avB = avp.tile([P, NC_CHUNK], F32, tag="av")
                    jmax = (c + 1) * NC_CHUNK // P - 1
                    for j in range(jmax + 1):
                        etA = attn_scores_exp(0, hp, c, j)
                        etB = attn_scores_exp(DH, hp, c, j)
                        attn_av(hA, avA, etA, c, j)
                        attn_av(hB, avB, etB, c, j)
                    attn_normalize(avA, OGT[0:DH, hp, ds(c * NC_CHUNK, NC_CHUNK)])
                    attn_normalize(avB, stg[:, ds(c * NC_CHUNK, NC_CHUNK)])
                # partition shift 0-63 -> 64-127 via sbuf-to-sbuf DMA
                nc.sync.dma_start(OGT[DH:P, hp, :], stg[:, :])

            # ---- o_proj partial: y[t, o] = sum_j ogT[j, t] * woT[j, o] ----
            for tt in range(NT):
                ps = psp.tile([P, D], F32, tag="s")
                for jt in range(DG // P):
                    _mm(
                        nc, ps,
                        lhsT=OGT[:, jt, ts(tt, P)],
                        rhs=WOT[:, jt, :],
                        start=(jt == 0), stop=(jt == DG // P - 1),
                    )
                ysb = work.tile([P, D], F32, tag="ysb")
                nc.vector.tensor_copy(ysb, ps)
                nc.sync.dma_start(y_d[ts(tt, P), :], ysb)

    # Pin Exp and Ln to the one table set holding both (same 400-piece
    # resolution); otherwise the table-load pass alternates exp_and_others /
    # natural_log, costing a ~1.4us ACT table load per softmax normalize.
    orig = bacc.get_activation_tables
    pref = "natural_log_exp_and_others"

    def tables_ln_exp_combined(arch):
        t = orig(arch)
        if pref in t:
            for name, funcs in t.items():
                if name != pref:
                    funcs.discard(mybir.ActivationFunctionType.Exp)
                    funcs.discard(mybir.ActivationFunctionType.Ln)
        return t

    bacc.get_activation_tables = tables_ln_exp_combined
    try:
        nc.compile()
    finally:
        bacc.get_activation_tables = orig
    return nc


def _get_nc():
    global _CACHED
    if _CACHED is None:
        _CACHED = _build_kernel()
    return _CACHED


def _shard_inputs(x, wq, wk, wv, wo):
    bf = ml_dtypes.bfloat16
    in_maps = []
    for core in range(8):
        b, g = divmod(core, 2)
        gs = slice(g * DG, (g + 1) * DG)
        in_maps.append({
            "xT": np.ascontiguousarray(x[b].T).astype(bf),
            "wqT": np.ascontiguousarray(wq[gs, :].T).astype(bf),
            "wkT": np.ascontiguousarray(wk[gs, :].T).astype(bf),
            "wvT": np.ascontiguousarray(wv[gs, :].T).astype(bf),
            "woT": np.ascontiguousarray(wo[:, gs].T).astype(bf),
        })
    return in_maps


def kernel(x, wq, wk, wv, wo, _trace=False, _trace_cores=None):
    x = np.asarray(x, dtype=np.float32)
    wq = np.asarray(wq, dtype=np.float32)
    wk = np.asarray(wk, dtype=np.float32)
    wv = np.asarray(wv, dtype=np.float32)
    wo = np.asarray(wo, dtype=np.float32)

    nc = _get_nc()
    in_maps = _shard_inputs(x, wq, wk, wv, wo)
    res = run_bass_kernel_spmd(
        nc, in_maps, core_ids=list(range(8)),
        trace=_trace,
        **({"trace_cores": _trace_cores} if _trace_cores else {}),
    )
    B = x.shape[0]
    y = np.zeros((B, T, D), dtype=np.float32)
    for core in range(8):
        b = core // 2
        y[b] += res.results[core]["y"]
    if _trace:
        return y, res
    return y
